# revision 2
# baseline (speedup 1.0000x reference)
"""Gemma4 patch-embed kernel for 8 Trainium2 NeuronCores.

Computation (see reference):
    x_re   = pixel-reorder(x)            # (ph,pw,C) -> (C,ph,pw) flat permutation of last dim
    x_in   = 2*(x_re - 0.5)
    proj   = einsum('bnk,dk->bnd', x_in, proj_w)
    pos    = pos_table[0][coord_x] + pos_table[1][coord_y]   (coords < 64)
    pos    = 0 where ~patch_valid
    out    = proj + pos
    also returns position_ids = patch_coord[..., ::-1], padding = ~patch_valid

Strategy:
  - Data parallel: batch b -> core b (8 batches, 4096 tokens each).
  - Host folds the pixel permutation into proj_w (W2 = 2*W_perm) and the
    "-1" affine shift into a per-channel bias (bias = -rowsum(W_perm)).
  - Host pre-transposes each x shard to [768, 4096] so the PE consumes it
    directly as the stationary operand (contraction dim on partitions).
  - Positional lookup runs as a one-hot matmul: coords are < 64, so both
    tables stack into one 128-row table; a K=2 broadcast matmul + DVE
    is_equal builds the masked one-hot (invalid tokens get coord -1 and
    produce an all-zero row). The gather matmul accumulates into the same
    PSUM tile as the projection matmul, so "proj + pos" is free.
  - Bias is added by DVE while evicting PSUM -> SBUF, then DMA to HBM.

The program can be built with an in-NEFF repeat loop (reps > 1) so tests
can wall-clock K iterations and recover per-iteration HW time by slope,
cancelling upload/dispatch overhead (no NTFF profiling on this client).
"""

import numpy as np

PH = PW = 16
B, N, D = 8, 4096, 768
NCORES = 8
VMAX = 64  # patch_coord values are in [0, 64)
P = 128
KC = D // P  # 6 k-chunks of 128
TG = 1024    # tokens per DMA group
NG = N // TG
TPG = TG // P  # 128-token tiles per group

_CACHE = {}


def _build_program(reps=1):
    from concourse import bacc
    import concourse.tile as tile
    import concourse.mybir as mybir

    nc = bacc.Bacc(None, target_bir_lowering=False, debug=False, num_devices=NCORES)
    f32 = mybir.dt.float32

    xT = nc.dram_tensor("xT", [D, N], f32, kind="ExternalInput").ap()
    cxy = nc.dram_tensor("cxy", [2, N], f32, kind="ExternalInput").ap()
    w2t = nc.dram_tensor("w2t", [D, D], f32, kind="ExternalInput").ap()
    tcat = nc.dram_tensor("tcat", [P, D], f32, kind="ExternalInput").ap()
    biasb = nc.dram_tensor("biasb", [P, D], f32, kind="ExternalInput").ap()
    sel = nc.dram_tensor("sel", [2, P], f32, kind="ExternalInput").ap()
    iota2 = nc.dram_tensor("iota2", [P, 1], f32, kind="ExternalInput").ap()
    y = nc.dram_tensor("y", [N, D], f32, kind="ExternalOutput").ap()

    with tile.TileContext(nc) as tc:
        with (
            tc.tile_pool(name="const", bufs=1) as cpool,
            tc.tile_pool(name="xin", bufs=2) as xpool,
            tc.tile_pool(name="oh", bufs=3) as ohpool,
            tc.tile_pool(name="yout", bufs=3) as ypool,
            tc.tile_pool(name="pout", bufs=2, space="PSUM") as poutpool,
            tc.tile_pool(name="pbc", bufs=2, space="PSUM") as pbcpool,
        ):
            w2t_sb = cpool.tile([P, KC, D], f32)
            nc.sync.dma_start(w2t_sb, w2t.rearrange("(o p) d -> p o d", p=P))
            tcat_sb = cpool.tile([P, D], f32)
            nc.sync.dma_start(tcat_sb, tcat)
            biasb_sb = cpool.tile([P, D], f32)
            nc.sync.dma_start(biasb_sb, biasb)
            sel_sb = cpool.tile([2, P], f32)
            nc.sync.dma_start(sel_sb, sel)
            iota_sb = cpool.tile([P, 1], f32)
            nc.sync.dma_start(iota_sb, iota2)
            cxy_sb = cpool.tile([2, N], f32)
            nc.sync.dma_start(cxy_sb, cxy)

            xT_r = xT.rearrange("(o p) t -> p o t", p=P)

            def body():
                for g in range(NG):
                    xt = xpool.tile([P, KC, TG], f32)
                    nc.sync.dma_start(xt, xT_r[:, :, g * TG:(g + 1) * TG])
                    for ti in range(TPG):
                        t0 = g * TG + ti * P
                        # broadcast coords across partitions:
                        # pbc[v, t] = cx'[t] for v<64 else cy'[t]
                        pbc = pbcpool.tile([P, P], f32)
                        nc.tensor.matmul(
                            pbc, sel_sb, cxy_sb[:, t0:t0 + P],
                            start=True, stop=True,
                        )
                        # masked one-hot: oh[v, t] = (v % 64 == pbc[v, t])
                        oh = ohpool.tile([P, P], f32)
                        nc.vector.tensor_scalar(
                            oh, pbc, iota_sb, None, mybir.AluOpType.is_equal
                        )
                        pout = poutpool.tile([P, D], f32)
                        for o in range(KC):
                            lhsT = xt[:, o, ti * P:(ti + 1) * P]
                            nc.tensor.matmul(
                                pout[:, 0:512], lhsT, w2t_sb[:, o, 0:512],
                                start=(o == 0), stop=False,
                            )
                            nc.tensor.matmul(
                                pout[:, 512:D], lhsT, w2t_sb[:, o, 512:D],
                                start=(o == 0), stop=False,
                            )
                        # positional gather accumulates into the same PSUM
                        nc.tensor.matmul(
                            pout[:, 0:512], oh, tcat_sb[:, 0:512],
                            start=False, stop=True,
                        )
                        nc.tensor.matmul(
                            pout[:, 512:D], oh, tcat_sb[:, 512:D],
                            start=False, stop=True,
                        )
                        yt = ypool.tile([P, D], f32)
                        nc.vector.tensor_tensor(
                            yt, pout, biasb_sb, mybir.AluOpType.add
                        )
                        nc.sync.dma_start(y[t0:t0 + P, :], yt)

            if reps == 1:
                body()
            else:
                with tc.For_i(0, reps, 1,
                              hint_engines=(mybir.EngineType.PE,)):
                    body()

    nc.compile()
    return nc


def _get_program(reps=1):
    key = ("nc", reps)
    if key not in _CACHE:
        _CACHE[key] = _build_program(reps)
    return _CACHE[key]


def _prep_host(x, patch_coord, patch_valid, proj_w, pos_table):
    # Fold the (ph,pw,C)->(C,ph,pw) pixel permutation into the weights:
    # x_re[j] = x[src[j]]  =>  Wp[:, src] = proj_w
    C = D // (PH * PW)
    j = np.arange(D)
    c, r = j // (PH * PW), j % (PH * PW)
    ph, pw = r // PW, r % PW
    src = ph * (PW * C) + pw * C + c
    Wp = np.empty_like(proj_w)
    Wp[:, src] = proj_w
    # out = (2x-1) @ Wp^T = x @ (2Wp)^T - rowsum(Wp)
    w2t = np.ascontiguousarray((2.0 * Wp).T)
    bias = (-Wp.sum(axis=1)).astype(np.float32)
    biasb = np.ascontiguousarray(np.broadcast_to(bias, (P, D)))

    tcat = np.ascontiguousarray(
        np.concatenate([pos_table[0, :VMAX], pos_table[1, :VMAX]], axis=0)
    ).astype(np.float32)

    selv = np.zeros((2, P), np.float32)
    selv[0, :VMAX] = 1.0
    selv[1, VMAX:] = 1.0
    iota2 = (np.arange(P, dtype=np.float32) % VMAX).reshape(P, 1)

    # per-token table indices, clamped like the reference, then masked to -1
    # for invalid tokens (no one-hot row matches -1 -> pos contribution 0)
    cx = np.maximum(patch_coord[..., 1], 0)  # position_ids[...,0]
    cy = np.maximum(patch_coord[..., 0], 0)  # position_ids[...,1]
    assert cx.max() < VMAX and cy.max() < VMAX, "coords exceed table slice"
    cxm = np.where(patch_valid, cx, -1).astype(np.float32)
    cym = np.where(patch_valid, cy, -1).astype(np.float32)

    in_maps = []
    for b in range(B):
        in_maps.append({
            "xT": np.ascontiguousarray(x[b].T),
            "cxy": np.ascontiguousarray(np.stack([cxm[b], cym[b]])),
            "w2t": w2t,
            "tcat": tcat,
            "biasb": biasb,
            "sel": selv,
            "iota2": iota2,
        })
    return in_maps


class Runner:
    """Jitted shard_map executable over the 8 cores with device-resident
    inputs (no donation), so repeated .run() calls measure execution only.
    Mirrors bass2jax.run_bass_via_pjrt's multi-core path."""

    def __init__(self, reps=1):
        import jax
        import concourse.mybir as mybir
        from concourse import bass2jax
        from jax.experimental.shard_map import shard_map
        from jax.sharding import Mesh, PartitionSpec, NamedSharding

        nc = _get_program(reps)
        bass2jax.install_neuronx_cc_hook()
        self.reps = reps
        self.nc = nc
        partition_name = (
            nc.partition_id_tensor.name if nc.partition_id_tensor else None
        )
        in_names, out_names, out_avals, zero_outs = [], [], [], []
        for alloc in nc.m.functions[0].allocations:
            if not isinstance(alloc, mybir.MemoryLocationSet):
                continue
            name = alloc.memorylocations[0].name
            if alloc.kind == "ExternalInput":
                if name != partition_name:
                    in_names.append(name)
            elif alloc.kind == "ExternalOutput":
                shape = tuple(alloc.tensor_shape)
                dtype = mybir.dt.np(alloc.dtype)
                out_names.append(name)
                out_avals.append(jax.core.ShapedArray(shape, dtype))
                zero_outs.append(np.zeros(shape, dtype))
        n_params = len(in_names)
        self.in_param_names = list(in_names)
        self.out_names = out_names
        in_names = in_names + out_names
        if partition_name is not None:
            in_names.append(partition_name)

        def _body(*args):
            operands = list(args)
            if partition_name is not None:
                operands.append(bass2jax.partition_id_tensor())
            outs = bass2jax._bass_exec_p.bind(
                *operands,
                out_avals=tuple(out_avals),
                in_names=tuple(in_names),
                out_names=tuple(out_names),
                lowering_input_output_aliases=(),
                sim_require_finite=True,
                sim_require_nnan=True,
                nc=nc,
            )
            return tuple(outs)

        devices = jax.devices()[:NCORES]
        mesh = Mesh(np.asarray(devices), ("core",))
        n_outs = len(out_names)
        self.fn = jax.jit(
            shard_map(
                _body,
                mesh=mesh,
                in_specs=(PartitionSpec("core"),) * (n_params + n_outs),
                out_specs=(PartitionSpec("core"),) * n_outs,
                check_rep=False,
            ),
            keep_unused=True,
        )
        self.sharding = NamedSharding(mesh, PartitionSpec("core"))
        self.zero_outs = zero_outs
        self.out_avals = out_avals
        self.args = None
        self._jax = jax

    def load(self, in_maps):
        jax = self._jax
        concat_in = [
            np.concatenate(
                [np.asarray(in_maps[c][name]) for c in range(NCORES)], axis=0
            )
            for name in self.in_param_names
        ]
        concat_zeros = [
            np.zeros((NCORES * z.shape[0], *z.shape[1:]), z.dtype)
            for z in self.zero_outs
        ]
        self.args = [
            jax.device_put(a, self.sharding) for a in concat_in + concat_zeros
        ]
        jax.block_until_ready(self.args)

    def run(self):
        outs = self.fn(*self.args)
        self._jax.block_until_ready(outs)
        return outs

    def results(self):
        outs = self.run()
        return [
            {
                name: np.asarray(outs[i]).reshape(
                    NCORES, *self.out_avals[i].shape
                )[c]
                for i, name in enumerate(self.out_names)
            }
            for c in range(NCORES)
        ]


def kernel(x, patch_coord, patch_valid, proj_w, pos_table):
    x = np.asarray(x, dtype=np.float32)
    patch_coord = np.asarray(patch_coord)
    patch_valid = np.asarray(patch_valid)
    proj_w = np.asarray(proj_w, dtype=np.float32)
    pos_table = np.asarray(pos_table, dtype=np.float32)

    in_maps = _prep_host(x, patch_coord, patch_valid, proj_w, pos_table)

    runner = _CACHE.get("runner")
    if runner is None:
        runner = _CACHE["runner"] = Runner(reps=1)
    runner.load(in_maps)
    res = runner.results()
    out = np.stack([res[b]["y"] for b in range(B)])

    position_ids = np.ascontiguousarray(patch_coord[..., ::-1])
    padding_positions = ~patch_valid
    return out, position_ids, padding_positions
